# revision 27
# baseline (speedup 1.0000x reference)
"""Trainium2 Bass kernel for dilated window attention (nn_Dilated_attn).

Strategy (8 NeuronCores, data-parallel over the 1024 dilated windows):
 - Host regroups x into (1024 windows, 64 tok, 512) and shards 128 windows/core.
 - RoPE is folded into the QKV weights: 8 row-rotation + 8 col-rotation weight
   variants per q/k half, applied via position-sliced GEMMs. With CH=32 windows
   per chunk each weight-variant matmul has a 256-wide moving operand, so
   LDWEIGHTS is hidden under compute and the PE stays HAM-warm.
 - q.T/k.T computed d-major then DMA-scattered so head h lives on partition
   strip 32*(h%4); scores for 4 heads run concurrently via PE row tiling
   (tile_position=(32*jj, 0), K=32 quadrants).
 - AV uses exp(S).T as the stationary with per-head [v_ws0|v_ws1|ones2]
   zero-padded moving blocks: one matmul per head yields attention output and
   softmax denominators; normalize via broadcast-AP multiply on DVE.
 - PE-transpose (2-group blocks, double-buffered) -> d-major proj GEMM
   per 4-group block -> bf16 DMA out.
"""

import sys
import numpy as np
import ml_dtypes

sys.path.insert(0, "/opt/trn_rl_repo")

import concourse.bass as bass  # noqa: E402
import concourse.tile as tile  # noqa: E402
from concourse import bacc, mybir  # noqa: E402
from concourse.masks import make_identity  # noqa: E402
from contextlib import ExitStack  # noqa: E402

# ---------------- problem constants ----------------
DIM = 512
HEADS = 16
HD = 32
WH, WW = 8, 8
D0, D1 = 2, 2
TWH, TWW = 16, 16
SCALE = HD ** -0.5
N_CORES = 8
NWIN = 1024
NTOK = 64
WIN_PER_CORE = NWIN // N_CORES      # 128
NT = WIN_PER_CORE * NTOK            # 8192 tokens per core

d_half = HD // 2                    # 16
INV = 1.0 / (10000.0 ** (np.arange(0, d_half, 2, dtype=np.float64) / d_half))

BF16 = mybir.dt.bfloat16
F32 = mybir.dt.float32


# ---------------- host-side data prep ----------------

def window_partition(x):
    B, H, W, C = x.shape
    xw = x.reshape(B, H // TWH, TWH, W // TWW, TWW, C).transpose(0, 1, 3, 2, 4, 5)
    xw = xw.reshape(-1, TWH, TWW, C)
    B_ = xw.shape[0]
    xw = xw.reshape(B_, TWH // D0, D0, TWW // D1, D1, C).transpose(0, 1, 3, 2, 4, 5)
    xw = xw.reshape(B_, WH * WW, D0 * D1, C)
    return xw.transpose(0, 2, 1, 3).reshape(B_ * D0 * D1, WH * WW, C)


def window_unpartition(ow, B, H, W):
    C = ow.shape[-1]
    B_ = ow.shape[0] // (D0 * D1)
    o = ow.reshape(B_, D0 * D1, WH * WW, C).transpose(0, 2, 1, 3)
    o = o.reshape(B_, WH, WW, D0, D1, C).transpose(0, 1, 3, 2, 4, 5)
    o = o.reshape(B_, TWH, TWW, C)
    nh, nw = H // TWH, W // TWW
    o = o.reshape(B, nh, nw, TWH, TWW, C).transpose(0, 1, 3, 2, 4, 5)
    return o.reshape(B, H, W, C)


PERM32 = np.concatenate([
    np.arange(0, d_half, 2), np.arange(1, d_half, 2),
    d_half + np.arange(0, d_half, 2), d_half + np.arange(1, d_half, 2),
])


def _rot_mat(theta_vec):
    c, s = np.cos(theta_vec), np.sin(theta_vec)
    R = np.zeros((16, 16))
    R[np.arange(8), np.arange(8)] = c
    R[np.arange(8), 8 + np.arange(8)] = -s
    R[8 + np.arange(8), np.arange(8)] = s
    R[8 + np.arange(8), 8 + np.arange(8)] = c
    return R


def prep_weights(qkv_w, qkv_b, proj_w, proj_b):
    qkv_w = np.asarray(qkv_w, np.float64)
    qkv_b = np.asarray(qkv_b, np.float64)
    proj_w = np.asarray(proj_w, np.float64)
    proj_b = np.asarray(proj_b, np.float64)

    Wq = qkv_w[:DIM] * SCALE
    Wk = qkv_w[DIM:2 * DIM]
    Wv = qkv_w[2 * DIM:]
    bq = qkv_b[:DIM] * SCALE
    bk = qkv_b[DIM:2 * DIM]
    bv = qkv_b[2 * DIM:]

    perm = (np.arange(HEADS)[:, None] * HD + PERM32[None, :]).reshape(-1)
    Wq_p, bq_p = Wq[perm], bq[perm]
    Wk_p, bk_p = Wk[perm], bk[perm]

    idx = np.arange(DIM).reshape(HEADS, HD)
    a_rows = idx[:, :16].reshape(-1)
    b_rows = idx[:, 16:].reshape(-1)

    def variants(Wp, bp, rows):
        Wh, bh = Wp[rows], bp[rows]
        Ws, bs = [], []
        for t in range(8):
            R = np.kron(np.eye(HEADS), _rot_mat(t * INV))
            Ws.append(R @ Wh)
            bs.append(R @ bh)
        return np.stack(Ws), np.stack(bs)

    Wqa, bqa = variants(Wq_p, bq_p, a_rows)
    Wqb, bqb = variants(Wq_p, bq_p, b_rows)
    Wka, bka = variants(Wk_p, bk_p, a_rows)
    Wkb, bkb = variants(Wk_p, bk_p, b_rows)

    proj_b_eff = proj_w @ bv + proj_b
    bias_zero = (np.abs(np.concatenate([bqa, bqb, bka, bkb], None)).max() == 0.0
                 and np.abs(proj_b_eff).max() == 0.0)

    def pack_lhsT(Wvar):  # (8, 256, 512) -> (8, 4, 128, 256) bf16
        WT = Wvar.transpose(0, 2, 1)                # (8, 512, 256)
        return np.ascontiguousarray(
            WT.reshape(8, 4, 128, 256)).astype(ml_dtypes.bfloat16)

    return dict(
        wqa=pack_lhsT(Wqa), wqb=pack_lhsT(Wqb),
        wka=pack_lhsT(Wka), wkb=pack_lhsT(Wkb),
        wv=np.ascontiguousarray(Wv.T.reshape(4, 128, 512)).astype(ml_dtypes.bfloat16),
        wp=np.ascontiguousarray(proj_w.T.reshape(4, 128, 512)).astype(ml_dtypes.bfloat16),
        proj_b_eff=proj_b_eff.astype(np.float32),
        bias_zero=bias_zero,
    )


# ---------------- device program ----------------

def build_program(CH=32, NCHUNK=4, debug_stop=None):
    """One-core SPMD program. CH windows per chunk, NCHUNK chunks.
    debug_stop: one of qkgemm|scatter|vprime|scores|av|trans to truncate
    the pipeline after that stage and DMA the stage output to a `dbg` tensor."""
    nt = CH * NCHUNK * NTOK        # tokens per core
    TPC = CH * NTOK                # tokens per chunk
    NG = CH // 2                   # 2-window groups per chunk

    nc = bacc.Bacc(trn_type="TRN2", target_bir_lowering=False, debug=False)

    # xtr: tokens ordered (r, w, c) per chunk -> a-half moving operands are
    # contiguous 256-col blocks. xtc: (c, w, r) order for the b-half.
    # Strided moving APs cost ~2x per column, so contiguity is critical.
    xtr_d = nc.dram_tensor("xtr", [4, 128, nt], BF16, kind="ExternalInput").ap()
    xtc_d = nc.dram_tensor("xtc", [4, 128, nt], BF16, kind="ExternalInput").ap()
    w_d = {}
    for nm in ("wqa", "wqb", "wka", "wkb"):
        w_d[nm] = nc.dram_tensor(nm, [8, 4, 128, 256], BF16,
                                 kind="ExternalInput").ap()
    wv_d = nc.dram_tensor("wv", [4, 128, 512], BF16, kind="ExternalInput").ap()
    wp_d = nc.dram_tensor("wp", [4, 128, 512], BF16, kind="ExternalInput").ap()
    out_d = nc.dram_tensor("outT", [4, 128, nt], BF16, kind="ExternalOutput").ap()

    with tile.TileContext(nc) as tc, ExitStack() as ctx:
        const_p = ctx.enter_context(tc.tile_pool(name="const", bufs=1))
        w_p = ctx.enter_context(tc.tile_pool(name="weights", bufs=1))
        xt_p = ctx.enter_context(tc.tile_pool(name="xt", bufs=2))
        stag_p = ctx.enter_context(tc.tile_pool(name="stag", bufs=2))
        qkT_p = ctx.enter_context(tc.tile_pool(name="qkT", bufs=1))
        vp_p = ctx.enter_context(tc.tile_pool(name="vp", bufs=1))
        exp_p = ctx.enter_context(tc.tile_pool(name="exp", bufs=3))
        ao_p = ctx.enter_context(tc.tile_pool(name="ao", bufs=4))
        aoT_p = ctx.enter_context(tc.tile_pool(name="aoT", bufs=2))
        rc_p = ctx.enter_context(tc.tile_pool(name="rc", bufs=2))
        os_p = ctx.enter_context(tc.tile_pool(name="os", bufs=2))
        vs_p = ctx.enter_context(tc.tile_pool(name="vs", bufs=3))

        ps_gemm = ctx.enter_context(tc.tile_pool(name="ps_gemm", bufs=2, space="PSUM"))
        ps_sc = ctx.enter_context(tc.tile_pool(name="ps_sc", bufs=1, space="PSUM"))
        ps_av = ctx.enter_context(tc.tile_pool(name="ps_av", bufs=2, space="PSUM"))

        ident = const_p.tile([128, 128], BF16)
        make_identity(nc, ident[:])

        # resident weights
        w_sb = {}
        for nm in ("wqa", "wqb", "wka", "wkb"):
            t = w_p.tile([128, 8, 4, 256], BF16, tag=nm)
            nc.sync.dma_start(out=t[:], in_=w_d[nm].rearrange("v k p m -> p v k m"))
            w_sb[nm] = t
        wv_sb = w_p.tile([128, 4, 512], BF16, tag="wv")
        nc.sync.dma_start(out=wv_sb[:], in_=wv_d.rearrange("k p n -> p k n"))
        wp_sb = w_p.tile([128, 4, 512], BF16, tag="wp")
        nc.sync.dma_start(out=wp_sb[:], in_=wp_d.rearrange("k p n -> p k n"))

        # persistent zero-padded v tiles (zeros/ones written once, data per g).
        # Per head h a 66-col block [v_ws0(32) | v_ws1(32) | ones(2)] so AV is
        # a single N=66 matmul per head (denominators fused via ones cols).
        vz_tiles = []
        for i in range(4):
            vzt = vp_p.tile([128, 16, 66], BF16, tag=f"vp{i}")
            nc.gpsimd.memset(vzt[64:128, :, 0:32], 0.0)
            nc.gpsimd.memset(vzt[0:64, :, 32:64], 0.0)
            nc.gpsimd.memset(vzt[:, :, 64:66], 0.0)
            nc.gpsimd.memset(vzt[0:64, :, 64:65], 1.0)
            nc.gpsimd.memset(vzt[64:128, :, 65:66], 1.0)
            vz_tiles.append(vzt)

        # persistent qT/kT tiles: head h = 4*hg + jj lives on partition strip
        # 32*jj (+16 for the b-half) at free block hg. Fixed 32-aligned strips
        # let scores run 4 heads concurrently via PE row tiling.
        qkT = {}
        for T in "qk":
            qkT[T] = qkT_p.tile([128, 4, TPC], BF16, tag=T + "T",
                                name=T + "T_tile")

        def emit_vgemm(ck, g):
            # stationary (weights) APs must be single-free-dim: gather the
            # group's 128 tokens from xtr into a contiguous scratch (gpsimd,
            # otherwise idle) before using them as the v-GEMM stationary.
            t0 = ck * TPC
            xr4 = xt_tiles[ck % 2][:].rearrange(
                "p k (r w c) -> p k r w c", r=8, w=CH)
            scr = vs_p.tile([128, 4, 128], BF16, tag="vscr")
            nc.gpsimd.tensor_copy(
                out=scr[:].rearrange("p k (w r c) -> p k r w c", w=2, r=8),
                in_=xr4[:, :, :, 2 * g:2 * g + 2, :])
            ps = ps_gemm.tile([128, 512], F32, tag="gemm")
            for kc in range(4):
                nc.tensor.matmul(
                    ps[:],
                    lhsT=scr[:, kc],
                    rhs=wv_sb[:, kc],
                    start=(kc == 0), stop=(kc == 3))
            psh = ps[:].rearrange("p (h e) -> p h e", h=16)
            vzg = vz_tiles[g % 4]
            nc.scalar.copy(out=vzg[0:64, :, 0:32], in_=psh[0:64])
            nc.vector.tensor_copy(out=vzg[64:128, :, 32:64], in_=psh[64:128])

        xt_tiles = {}

        for ck in range(NCHUNK):
            t0 = ck * TPC

            # ---- load x.T chunk (both token orders) ----
            xt_t = xt_p.tile([128, 4, TPC], BF16, tag="xtr")
            xt_tiles[ck % 2] = xt_t
            nc.sync.dma_start(
                out=xt_t[:],
                in_=xtr_d[:, :, t0:t0 + TPC].rearrange("k p t -> p k t"))
            xtc_t = xt_p.tile([128, 4, TPC], BF16, tag="xtc", bufs=1)
            nc.gpsimd.dma_start(
                out=xtc_t[:],
                in_=xtc_d[:, :, t0:t0 + TPC].rearrange("k p t -> p k t"))

            # ---- q/k GEMMs (rope folded) into staging, then scatter ----
            # Moving operand for variant v8 is a contiguous 256-col block of
            # xtr (a-half, token order r,w,c) or xtc (b-half, order c,w,r).
            for Ti, T in enumerate("qk"):
                for Hi, half in enumerate("ab"):
                    wt = w_sb["w" + T + half]
                    xsrc = xt_t if half == "a" else xtc_t
                    stag = stag_p.tile([128, 2, TPC], BF16, tag="stag")
                    for Mc in range(2):
                        if half == "a":
                            dst4 = stag[:, Mc].rearrange(
                                "p (w r c) -> p r w c", w=CH, r=8, c=8)
                        else:
                            dst4 = stag[:, Mc].rearrange(
                                "p (w r c) -> p c w r", w=CH, r=8, c=8)
                        NW = CH * 8
                        for vp in range(4):
                            ps = ps_gemm.tile([128, 512], F32, tag="gemm")
                            for vv in range(2):
                                v8 = 2 * vp + vv
                                for kc in range(4):
                                    nc.tensor.matmul(
                                        ps[:, NW * vv:NW * (vv + 1)],
                                        lhsT=wt[:, v8, kc, 128 * Mc:128 * Mc + 128],
                                        rhs=xsrc[:, kc,
                                                 NW * v8:NW * (v8 + 1)],
                                        start=(vv == 0 and kc == 0),
                                        stop=(vv == 1 and kc == 3))
                            # 1-elem-run strided writes (b-half) are ~4x slow
                            # on ScalarE; route them to VectorE instead.
                            csrc = ps[:, 0:2 * NW].rearrange(
                                "p (v w c) -> p v w c", v=2, w=CH)
                            if half == "a":
                                nc.scalar.copy(
                                    out=dst4[:, 2 * vp:2 * vp + 2], in_=csrc)
                            else:
                                nc.vector.tensor_copy(
                                    out=dst4[:, 2 * vp:2 * vp + 2], in_=csrc)

                    # scatter this (T, half) into qkT strips
                    qT = qkT[T]
                    for hl in range(8):
                        src = stag[16 * hl:16 * hl + 16, :, :]
                        dstp = 32 * (hl % 4) + 16 * Hi
                        dst = qT[dstp:dstp + 16].rearrange(
                            "p (m hg) t -> p m hg t", m=2)[:, :, hl // 4, :]
                        eng = nc.sync if (hl % 2 == 0) else nc.gpsimd
                        eng.dma_start(out=dst, in_=src)
            if debug_stop == "scatter":
                dbg = nc.dram_tensor("dbg", [2, 128, 4, TPC], BF16,
                                     kind="ExternalOutput").ap()
                nc.sync.dma_start(out=dbg[0], in_=qkT["q"][:])
                nc.sync.dma_start(out=dbg[1], in_=qkT["k"][:])
                break

            # ---- per 2-window group: v GEMM + attention ----
            # Scores for 4 heads run concurrently on the 4 PE row strips;
            # each row-tile must drain to its OWN psum bank (same-bank
            # concurrent drains are a fatal HW error), hence the
            # [128, 4, 512] 4-bank scores tile. v-GEMM matmuls are
            # interleaved between attention groups so the PE keeps ahead
            # of the exp (ACT) latency on the shared scores banks.
            ao_tiles = {}
            aoTb = None
            for g in range(NG):
                if g == 0:
                    emit_vgemm(ck, 0)
                    emit_vgemm(ck, 1)
                vzg = vz_tiles[g % 4]
                cols = slice(128 * g, 128 * (g + 1))

                rcpg = rc_p.tile([128, 4, 4, 2], F32, tag="rc")
                aog = ao_p.tile([128, 512], BF16, tag="ao")
                ao_tiles[g % 2] = aog

                vnext = None
                vscr_cur = None
                if g + 2 < NG:
                    xr4 = xt_tiles[ck % 2][:].rearrange(
                        "p k (r w c) -> p k r w c", r=8, w=CH)
                    vscr_cur = vs_p.tile([128, 4, 128], BF16, tag="vscr")
                    nc.gpsimd.tensor_copy(
                        out=vscr_cur[:].rearrange(
                            "p k (w r c) -> p k r w c", w=2, r=8),
                        in_=xr4[:, :, :, 2 * (g + 2):2 * (g + 2) + 2, :])
                    vnext = ps_gemm.tile([128, 512], F32, tag="gemm")

                def emit_av(G4, expg):
                    ps_a = ps_av.tile([128, 4, 66], F32, tag="av")
                    for jj in range(4):
                        h = 4 * G4 + jj
                        nc.tensor.matmul(
                            ps_a[:, jj],
                            lhsT=expg[:, jj],
                            rhs=vzg[:, h, :],
                            start=(jj == 0), stop=(jj == 3))
                    # normalize: recip of denominators, scale valid halves
                    nc.vector.reciprocal(
                        out=rcpg[:, G4], in_=ps_a[:, :, 64:66])
                    for ws in range(2):
                        src = ps_a[64 * ws:64 * ws + 64, :, 32 * ws:32 * ws + 32]
                        rin = rcpg[64 * ws:64 * ws + 64, G4, :, ws:ws + 1] \
                            .broadcast_to((64, 4, 32))
                        dst = aog[64 * ws:64 * ws + 64,
                                  128 * G4:128 * (G4 + 1)].rearrange(
                            "p (j e) -> p j e", j=4)
                        nc.vector.tensor_tensor(
                            out=dst, in0=src, in1=rin, op=mybir.AluOpType.mult)

                exp_tiles = {}
                for G4 in range(4):
                    # scores.T: 4 heads concurrently on 4 PE row strips,
                    # each into its own psum bank (cols 0:128 of bank jj)
                    ps_s = ps_sc.tile([128, 4, 512], F32, tag="sc")
                    expg = exp_p.tile([128, 4, 128], BF16, tag="exp")
                    exp_tiles[G4] = expg
                    for jj in range(4):
                        nc.tensor.matmul(
                            ps_s[:, jj, 0:128],
                            lhsT=qkT["k"][32 * jj:32 * jj + 32, G4, cols],
                            rhs=qkT["q"][32 * jj:32 * jj + 32, G4, cols],
                            start=True, stop=True,
                            tile_position=(32 * jj, 0))
                    nc.scalar.activation(
                        out=expg[:], in_=ps_s[:, :, 0:128],
                        func=mybir.ActivationFunctionType.Exp)
                    # interleave one v-GEMM matmul of group g+2 as PE filler
                    if vnext is not None:
                        kc = G4
                        nc.tensor.matmul(
                            vnext[:],
                            lhsT=vscr_cur[:, kc],
                            rhs=wv_sb[:, kc],
                            start=(kc == 0), stop=(kc == 3))
                    if G4 > 0:
                        emit_av(G4 - 1, exp_tiles[G4 - 1])
                emit_av(3, exp_tiles[3])
                if vnext is not None:
                    psh = vnext[:].rearrange("p (h e) -> p h e", h=16)
                    vzn = vz_tiles[(g + 2) % 4]
                    nc.scalar.copy(out=vzn[0:64, :, 0:32], in_=psh[0:64])
                    nc.vector.tensor_copy(out=vzn[64:128, :, 32:64],
                                          in_=psh[64:128])

                # ---- transpose 2-group blocks -> d-major ----
                if g % 2 == 1:
                    if g % 4 == 1:
                        aoTb = aoT_p.tile([128, 4, 512], BF16, tag="aoT")
                    ps_t = ps_gemm.tile([128, 512], F32, tag="gemm")
                    ps_tv = ps_t[:].bitcast(BF16).rearrange(
                        "p (gg m t) -> p gg m t", gg=2, m=4)
                    for gg in range(2):
                        for m in range(4):
                            nc.tensor.transpose(
                                ps_tv[:, gg, m],
                                ao_tiles[gg][:, 128 * m:128 * (m + 1)],
                                ident[:])
                    half = (g % 4) // 2
                    nc.vector.tensor_copy(
                        out=aoTb[:, :, 256 * half:256 * half + 256].rearrange(
                            "p m (gg t) -> p gg m t", gg=2),
                        in_=ps_tv)

                # ---- proj GEMM per 4-group block + store ----
                if g % 4 == 3:
                    th = g // 4
                    for Mc in range(4):
                        ps_o = ps_gemm.tile([128, 512], F32, tag="gemm")
                        for m in range(4):
                            nc.tensor.matmul(
                                ps_o[:],
                                lhsT=wp_sb[:, m, 128 * Mc:128 * Mc + 128],
                                rhs=aoTb[:, m, :],
                                start=(m == 0), stop=(m == 3))
                        ost = os_p.tile([128, 512], BF16, tag="os")
                        if Mc % 2 == 0:
                            nc.vector.tensor_copy(out=ost[:], in_=ps_o[:])
                        else:
                            nc.scalar.copy(out=ost[:], in_=ps_o[:])
                        eng = nc.sync if (Mc % 2 == 0) else nc.gpsimd
                        eng.dma_start(
                            out=out_d[Mc, :, t0 + 512 * th:t0 + 512 * (th + 1)],
                            in_=ost[:])

    nc.compile()
    return nc


# ---------------- host driver ----------------

_PROG_CACHE = {}


def _get_program(CH=32, NCHUNK=4):
    key = (CH, NCHUNK)
    if key not in _PROG_CACHE:
        _PROG_CACHE[key] = build_program(CH, NCHUNK)
    return _PROG_CACHE[key]


def make_in_maps(x, wp_dict, CH=32, NCHUNK=4, n_cores=N_CORES):
    xw = window_partition(np.asarray(x, np.float32))     # (1024, 64, 512)
    nt = CH * NCHUNK * NTOK
    win_per_core = nt // NTOK
    in_maps = []
    for c in range(n_cores):
        xs = xw[c * win_per_core:(c + 1) * win_per_core].reshape(nt, DIM)
        # token reorders per chunk: xtr = (r, w, c), xtc = (c, w, r)
        x5 = xs.reshape(NCHUNK, CH, 8, 8, DIM)           # (ck, w, r, c, D)
        xr = x5.transpose(0, 2, 1, 3, 4).reshape(nt, DIM)
        xc = x5.transpose(0, 3, 1, 2, 4).reshape(nt, DIM)
        xtr = np.ascontiguousarray(xr.T).astype(ml_dtypes.bfloat16)
        xtc = np.ascontiguousarray(xc.T).astype(ml_dtypes.bfloat16)
        in_maps.append(dict(
            xtr=np.ascontiguousarray(xtr.reshape(4, 128, nt)),
            xtc=np.ascontiguousarray(xtc.reshape(4, 128, nt)),
            wqa=wp_dict["wqa"], wqb=wp_dict["wqb"],
            wka=wp_dict["wka"], wkb=wp_dict["wkb"],
            wv=wp_dict["wv"], wp=wp_dict["wp"],
        ))
    return in_maps


def _run(x, qkv_w, qkv_b, proj_w, proj_b, trace=False):
    from concourse.bass_utils import run_bass_kernel_spmd

    wp_dict = prep_weights(qkv_w, qkv_b, proj_w, proj_b)
    assert wp_dict["bias_zero"], "nonzero biases not supported by this kernel"

    nc = _get_program()
    in_maps = make_in_maps(x, wp_dict)
    res = run_bass_kernel_spmd(nc, in_maps, list(range(N_CORES)), trace=trace)

    x = np.asarray(x)
    B, H, W, C = x.shape
    outs = []
    for c in range(N_CORES):
        oT = np.asarray(res.results[c]["outT"]).astype(np.float32).reshape(DIM, NT)
        outs.append(np.ascontiguousarray(oT.T))          # (8192, 512)
    ow = np.concatenate(outs, 0).reshape(NWIN, NTOK, DIM)
    out = window_unpartition(ow, B, H, W).astype(np.float32)
    return out, res


def kernel(x, qkv_w, qkv_b, proj_w, proj_b):
    out, _ = _run(x, qkv_w, qkv_b, proj_w, proj_b, trace=False)
    return out


if __name__ == "__main__":
    build_program(4, 2)
    print("mini program built OK")
    build_program()
    print("full program built OK")


# revision 28
# speedup vs baseline: 1.1101x; 1.1101x over previous
"""Trainium2 Bass kernel for dilated window attention (nn_Dilated_attn).

Strategy (8 NeuronCores, data-parallel over the 1024 dilated windows):
 - Host regroups x into (1024 windows, 64 tok, 512) and shards 128 windows/core.
 - RoPE is folded into the QKV weights: 8 row-rotation + 8 col-rotation weight
   variants per q/k half, applied via position-sliced GEMMs. With CH=32 windows
   per chunk each weight-variant matmul has a 256-wide moving operand, so
   LDWEIGHTS is hidden under compute and the PE stays HAM-warm.
 - q.T/k.T computed d-major then DMA-scattered so head h lives on partition
   strip 32*(h%4); scores for 4 heads run concurrently via PE row tiling
   (tile_position=(32*jj, 0), K=32 quadrants).
 - AV uses exp(S).T as the stationary with per-head [v_ws0|v_ws1|ones2]
   zero-padded moving blocks: one matmul per head yields attention output and
   softmax denominators; normalize via broadcast-AP multiply on DVE.
 - PE-transpose (2-group blocks, double-buffered) -> d-major proj GEMM
   per 4-group block -> bf16 DMA out.
"""

import sys
import numpy as np
import ml_dtypes

sys.path.insert(0, "/opt/trn_rl_repo")

import concourse.bass as bass  # noqa: E402
import concourse.tile as tile  # noqa: E402
from concourse import bacc, mybir  # noqa: E402
from concourse.masks import make_identity  # noqa: E402
from contextlib import ExitStack  # noqa: E402

# ---------------- problem constants ----------------
DIM = 512
HEADS = 16
HD = 32
WH, WW = 8, 8
D0, D1 = 2, 2
TWH, TWW = 16, 16
SCALE = HD ** -0.5
N_CORES = 8
NWIN = 1024
NTOK = 64
WIN_PER_CORE = NWIN // N_CORES      # 128
NT = WIN_PER_CORE * NTOK            # 8192 tokens per core

d_half = HD // 2                    # 16
INV = 1.0 / (10000.0 ** (np.arange(0, d_half, 2, dtype=np.float64) / d_half))

BF16 = mybir.dt.bfloat16
F32 = mybir.dt.float32


# ---------------- host-side data prep ----------------

def window_partition(x):
    B, H, W, C = x.shape
    xw = x.reshape(B, H // TWH, TWH, W // TWW, TWW, C).transpose(0, 1, 3, 2, 4, 5)
    xw = xw.reshape(-1, TWH, TWW, C)
    B_ = xw.shape[0]
    xw = xw.reshape(B_, TWH // D0, D0, TWW // D1, D1, C).transpose(0, 1, 3, 2, 4, 5)
    xw = xw.reshape(B_, WH * WW, D0 * D1, C)
    return xw.transpose(0, 2, 1, 3).reshape(B_ * D0 * D1, WH * WW, C)


def window_unpartition(ow, B, H, W):
    C = ow.shape[-1]
    B_ = ow.shape[0] // (D0 * D1)
    o = ow.reshape(B_, D0 * D1, WH * WW, C).transpose(0, 2, 1, 3)
    o = o.reshape(B_, WH, WW, D0, D1, C).transpose(0, 1, 3, 2, 4, 5)
    o = o.reshape(B_, TWH, TWW, C)
    nh, nw = H // TWH, W // TWW
    o = o.reshape(B, nh, nw, TWH, TWW, C).transpose(0, 1, 3, 2, 4, 5)
    return o.reshape(B, H, W, C)


PERM32 = np.concatenate([
    np.arange(0, d_half, 2), np.arange(1, d_half, 2),
    d_half + np.arange(0, d_half, 2), d_half + np.arange(1, d_half, 2),
])


def _rot_mat(theta_vec):
    c, s = np.cos(theta_vec), np.sin(theta_vec)
    R = np.zeros((16, 16))
    R[np.arange(8), np.arange(8)] = c
    R[np.arange(8), 8 + np.arange(8)] = -s
    R[8 + np.arange(8), np.arange(8)] = s
    R[8 + np.arange(8), 8 + np.arange(8)] = c
    return R


def prep_weights(qkv_w, qkv_b, proj_w, proj_b):
    qkv_w = np.asarray(qkv_w, np.float64)
    qkv_b = np.asarray(qkv_b, np.float64)
    proj_w = np.asarray(proj_w, np.float64)
    proj_b = np.asarray(proj_b, np.float64)

    Wq = qkv_w[:DIM] * SCALE
    Wk = qkv_w[DIM:2 * DIM]
    Wv = qkv_w[2 * DIM:]
    bq = qkv_b[:DIM] * SCALE
    bk = qkv_b[DIM:2 * DIM]
    bv = qkv_b[2 * DIM:]

    perm = (np.arange(HEADS)[:, None] * HD + PERM32[None, :]).reshape(-1)
    Wq_p, bq_p = Wq[perm], bq[perm]
    Wk_p, bk_p = Wk[perm], bk[perm]

    idx = np.arange(DIM).reshape(HEADS, HD)
    a_rows = idx[:, :16].reshape(-1)
    b_rows = idx[:, 16:].reshape(-1)

    def variants(Wp, bp, rows):
        Wh, bh = Wp[rows], bp[rows]
        Ws, bs = [], []
        for t in range(8):
            R = np.kron(np.eye(HEADS), _rot_mat(t * INV))
            Ws.append(R @ Wh)
            bs.append(R @ bh)
        return np.stack(Ws), np.stack(bs)

    Wqa, bqa = variants(Wq_p, bq_p, a_rows)
    Wqb, bqb = variants(Wq_p, bq_p, b_rows)
    Wka, bka = variants(Wk_p, bk_p, a_rows)
    Wkb, bkb = variants(Wk_p, bk_p, b_rows)

    proj_b_eff = proj_w @ bv + proj_b
    bias_zero = (np.abs(np.concatenate([bqa, bqb, bka, bkb], None)).max() == 0.0
                 and np.abs(proj_b_eff).max() == 0.0)

    def pack_lhsT(Wvar):  # (8, 256, 512) -> (8, 4, 128, 256) bf16
        WT = Wvar.transpose(0, 2, 1)                # (8, 512, 256)
        return np.ascontiguousarray(
            WT.reshape(8, 4, 128, 256)).astype(ml_dtypes.bfloat16)

    return dict(
        wqa=pack_lhsT(Wqa), wqb=pack_lhsT(Wqb),
        wka=pack_lhsT(Wka), wkb=pack_lhsT(Wkb),
        wv=np.ascontiguousarray(Wv.T.reshape(4, 128, 512)).astype(ml_dtypes.bfloat16),
        wp=np.ascontiguousarray(proj_w.T.reshape(4, 128, 512)).astype(ml_dtypes.bfloat16),
        proj_b_eff=proj_b_eff.astype(np.float32),
        bias_zero=bias_zero,
    )


# ---------------- device program ----------------

def build_program(CH=32, NCHUNK=4, debug_stop=None):
    """One-core SPMD program. CH windows per chunk, NCHUNK chunks.
    debug_stop: one of qkgemm|scatter|vprime|scores|av|trans to truncate
    the pipeline after that stage and DMA the stage output to a `dbg` tensor."""
    nt = CH * NCHUNK * NTOK        # tokens per core
    TPC = CH * NTOK                # tokens per chunk
    NG = CH // 2                   # 2-window groups per chunk

    nc = bacc.Bacc(trn_type="TRN2", target_bir_lowering=False, debug=False)

    # xtr: tokens ordered (r, w, c) per chunk -> a-half moving operands are
    # contiguous 256-col blocks. xtc: (c, w, r) order for the b-half.
    # Strided moving APs cost ~2x per column, so contiguity is critical.
    xtr_d = nc.dram_tensor("xtr", [4, 128, nt], BF16, kind="ExternalInput").ap()
    xtc_d = nc.dram_tensor("xtc", [4, 128, nt], BF16, kind="ExternalInput").ap()
    w_d = {}
    for nm in ("wqa", "wqb", "wka", "wkb"):
        w_d[nm] = nc.dram_tensor(nm, [8, 4, 128, 256], BF16,
                                 kind="ExternalInput").ap()
    wv_d = nc.dram_tensor("wv", [4, 128, 512], BF16, kind="ExternalInput").ap()
    wp_d = nc.dram_tensor("wp", [4, 128, 512], BF16, kind="ExternalInput").ap()
    out_d = nc.dram_tensor("outT", [4, 128, nt], BF16, kind="ExternalOutput").ap()

    with tile.TileContext(nc) as tc, ExitStack() as ctx:
        const_p = ctx.enter_context(tc.tile_pool(name="const", bufs=1))
        w_p = ctx.enter_context(tc.tile_pool(name="weights", bufs=1))
        xt_p = ctx.enter_context(tc.tile_pool(name="xt", bufs=2))
        stag_p = ctx.enter_context(tc.tile_pool(name="stag", bufs=2))
        qkT_p = ctx.enter_context(tc.tile_pool(name="qkT", bufs=1))
        vp_p = ctx.enter_context(tc.tile_pool(name="vp", bufs=1))
        exp_p = ctx.enter_context(tc.tile_pool(name="exp", bufs=3))
        ao_p = ctx.enter_context(tc.tile_pool(name="ao", bufs=4))
        aoT_p = ctx.enter_context(tc.tile_pool(name="aoT", bufs=2))
        rc_p = ctx.enter_context(tc.tile_pool(name="rc", bufs=2))
        os_p = ctx.enter_context(tc.tile_pool(name="os", bufs=2))
        vs_p = ctx.enter_context(tc.tile_pool(name="vs", bufs=3))

        ps_gemm = ctx.enter_context(tc.tile_pool(name="ps_gemm", bufs=2, space="PSUM"))
        ps_sc = ctx.enter_context(tc.tile_pool(name="ps_sc", bufs=1, space="PSUM"))
        ps_av = ctx.enter_context(tc.tile_pool(name="ps_av", bufs=2, space="PSUM"))

        ident = const_p.tile([128, 128], BF16)
        make_identity(nc, ident[:])

        # resident weights
        w_sb = {}
        for nm in ("wqa", "wqb", "wka", "wkb"):
            t = w_p.tile([128, 8, 4, 256], BF16, tag=nm)
            nc.sync.dma_start(out=t[:], in_=w_d[nm].rearrange("v k p m -> p v k m"))
            w_sb[nm] = t
        wv_sb = w_p.tile([128, 4, 512], BF16, tag="wv")
        nc.sync.dma_start(out=wv_sb[:], in_=wv_d.rearrange("k p n -> p k n"))
        wp_sb = w_p.tile([128, 4, 512], BF16, tag="wp")
        nc.sync.dma_start(out=wp_sb[:], in_=wp_d.rearrange("k p n -> p k n"))

        # persistent zero-padded v tiles (zeros/ones written once, data per g).
        # Per head h a 66-col block [v_ws0(32) | v_ws1(32) | ones(2)] so AV is
        # a single N=66 matmul per head (denominators fused via ones cols).
        vz_tiles = []
        for i in range(4):
            vzt = vp_p.tile([128, 16, 66], BF16, tag=f"vp{i}")
            nc.gpsimd.memset(vzt[64:128, :, 0:32], 0.0)
            nc.gpsimd.memset(vzt[0:64, :, 32:64], 0.0)
            nc.gpsimd.memset(vzt[:, :, 64:66], 0.0)
            nc.gpsimd.memset(vzt[0:64, :, 64:65], 1.0)
            nc.gpsimd.memset(vzt[64:128, :, 65:66], 1.0)
            vz_tiles.append(vzt)

        # persistent qT/kT tiles: head h = 4*hg + jj lives on partition strip
        # 32*jj (+16 for the b-half) at free block hg. Fixed 32-aligned strips
        # let scores run 4 heads concurrently via PE row tiling.
        qkT = {}
        for T in "qk":
            qkT[T] = qkT_p.tile([128, 4, TPC], BF16, tag=T + "T",
                                name=T + "T_tile")

        def emit_vgemm(ck, g):
            # stationary (weights) APs must be single-free-dim: gather the
            # group's 128 tokens from xtr into a contiguous scratch (gpsimd,
            # otherwise idle) before using them as the v-GEMM stationary.
            t0 = ck * TPC
            xr4 = xt_tiles[ck % 2][:].rearrange(
                "p k (r w c) -> p k r w c", r=8, w=CH)
            scr = vs_p.tile([128, 4, 128], BF16, tag="vscr")
            nc.gpsimd.tensor_copy(
                out=scr[:].rearrange("p k (w r c) -> p k r w c", w=2, r=8),
                in_=xr4[:, :, :, 2 * g:2 * g + 2, :])
            ps = ps_gemm.tile([128, 512], F32, tag="gemm")
            for kc in range(4):
                nc.tensor.matmul(
                    ps[:],
                    lhsT=scr[:, kc],
                    rhs=wv_sb[:, kc],
                    start=(kc == 0), stop=(kc == 3))
            psh = ps[:].rearrange("p (h e) -> p h e", h=16)
            vzg = vz_tiles[g % 4]
            nc.scalar.copy(out=vzg[0:64, :, 0:32], in_=psh[0:64])
            nc.vector.tensor_copy(out=vzg[64:128, :, 32:64], in_=psh[64:128])

        xt_tiles = {}

        for ck in range(NCHUNK):
            t0 = ck * TPC

            # ---- load x.T chunk (both token orders) ----
            xt_t = xt_p.tile([128, 4, TPC], BF16, tag="xtr")
            xt_tiles[ck % 2] = xt_t
            nc.sync.dma_start(
                out=xt_t[:],
                in_=xtr_d[:, :, t0:t0 + TPC].rearrange("k p t -> p k t"))
            xtc_t = xt_p.tile([128, 4, TPC], BF16, tag="xtc", bufs=1)
            nc.gpsimd.dma_start(
                out=xtc_t[:],
                in_=xtc_d[:, :, t0:t0 + TPC].rearrange("k p t -> p k t"))

            # ---- q/k GEMMs (rope folded) into staging, then scatter ----
            # Moving operand for variant v8 is a contiguous 256-col block of
            # xtr (a-half, token order r,w,c) or xtc (b-half, order c,w,r).
            for Ti, T in enumerate("qk"):
                for Hi, half in enumerate("ab"):
                    wt = w_sb["w" + T + half]
                    xsrc = xt_t if half == "a" else xtc_t
                    stag = stag_p.tile([128, 2, TPC], BF16, tag="stag")
                    for Mc in range(2):
                        if half == "a":
                            dst4 = stag[:, Mc].rearrange(
                                "p (w r c) -> p r w c", w=CH, r=8, c=8)
                        else:
                            dst4 = stag[:, Mc].rearrange(
                                "p (w r c) -> p c w r", w=CH, r=8, c=8)
                        NW = CH * 8
                        for vp in range(4):
                            ps = ps_gemm.tile([128, 512], F32, tag="gemm")
                            for vv in range(2):
                                v8 = 2 * vp + vv
                                for kc in range(4):
                                    nc.tensor.matmul(
                                        ps[:, NW * vv:NW * (vv + 1)],
                                        lhsT=wt[:, v8, kc, 128 * Mc:128 * Mc + 128],
                                        rhs=xsrc[:, kc,
                                                 NW * v8:NW * (v8 + 1)],
                                        start=(vv == 0 and kc == 0),
                                        stop=(vv == 1 and kc == 3))
                            if half == "a":
                                # dst runs of 8 (c contiguous): fine on ACT
                                nc.scalar.copy(
                                    out=dst4[:, 2 * vp:2 * vp + 2],
                                    in_=ps[:, 0:2 * NW].rearrange(
                                        "p (v w c) -> p v w c", v=2, w=CH))
                            else:
                                # b-half dst is c-strided; put the variant
                                # pair innermost (2-elem runs instead of 1)
                                # and alternate engines across vp pairs.
                                bdst = stag[:, Mc].rearrange(
                                    "p (w r c) -> p w r c", w=CH, r=8)[
                                    :, :, :, 2 * vp:2 * vp + 2]
                                bsrc = ps[:, 0:2 * NW].rearrange(
                                    "p (v w r) -> p w r v", v=2, w=CH)
                                if vp % 2 == 0:
                                    nc.vector.tensor_copy(out=bdst, in_=bsrc)
                                else:
                                    nc.scalar.copy(out=bdst, in_=bsrc)

                    # scatter this (T, half) into qkT strips
                    qT = qkT[T]
                    for hl in range(8):
                        src = stag[16 * hl:16 * hl + 16, :, :]
                        dstp = 32 * (hl % 4) + 16 * Hi
                        dst = qT[dstp:dstp + 16].rearrange(
                            "p (m hg) t -> p m hg t", m=2)[:, :, hl // 4, :]
                        eng = nc.sync if (hl % 2 == 0) else nc.gpsimd
                        eng.dma_start(out=dst, in_=src)
            if debug_stop == "scatter":
                dbg = nc.dram_tensor("dbg", [2, 128, 4, TPC], BF16,
                                     kind="ExternalOutput").ap()
                nc.sync.dma_start(out=dbg[0], in_=qkT["q"][:])
                nc.sync.dma_start(out=dbg[1], in_=qkT["k"][:])
                break

            # ---- per 2-window group: v GEMM + attention ----
            # Scores for 4 heads run concurrently on the 4 PE row strips;
            # each row-tile must drain to its OWN psum bank (same-bank
            # concurrent drains are a fatal HW error), hence the
            # [128, 4, 512] 4-bank scores tile. v-GEMM matmuls are
            # interleaved between attention groups so the PE keeps ahead
            # of the exp (ACT) latency on the shared scores banks.
            ao_tiles = {}
            aoTb = None
            for g in range(NG):
                if g == 0:
                    emit_vgemm(ck, 0)
                    emit_vgemm(ck, 1)
                vzg = vz_tiles[g % 4]
                cols = slice(128 * g, 128 * (g + 1))

                rcpg = rc_p.tile([128, 4, 4, 2], F32, tag="rc")
                aog = ao_p.tile([128, 512], BF16, tag="ao")
                ao_tiles[g % 2] = aog

                vnext = None
                vscr_cur = None
                if g + 2 < NG:
                    xr4 = xt_tiles[ck % 2][:].rearrange(
                        "p k (r w c) -> p k r w c", r=8, w=CH)
                    vscr_cur = vs_p.tile([128, 4, 128], BF16, tag="vscr")
                    nc.gpsimd.tensor_copy(
                        out=vscr_cur[:].rearrange(
                            "p k (w r c) -> p k r w c", w=2, r=8),
                        in_=xr4[:, :, :, 2 * (g + 2):2 * (g + 2) + 2, :])
                    vnext = ps_gemm.tile([128, 512], F32, tag="gemm")

                def emit_av(G4, expg):
                    ps_a = ps_av.tile([128, 4, 66], F32, tag="av")
                    for jj in range(4):
                        h = 4 * G4 + jj
                        nc.tensor.matmul(
                            ps_a[:, jj],
                            lhsT=expg[:, jj],
                            rhs=vzg[:, h, :],
                            start=(jj == 0), stop=(jj == 3))
                    # normalize: recip of denominators, scale valid halves
                    nc.vector.reciprocal(
                        out=rcpg[:, G4], in_=ps_a[:, :, 64:66])
                    for ws in range(2):
                        src = ps_a[64 * ws:64 * ws + 64, :, 32 * ws:32 * ws + 32]
                        rin = rcpg[64 * ws:64 * ws + 64, G4, :, ws:ws + 1] \
                            .broadcast_to((64, 4, 32))
                        dst = aog[64 * ws:64 * ws + 64,
                                  128 * G4:128 * (G4 + 1)].rearrange(
                            "p (j e) -> p j e", j=4)
                        nc.vector.tensor_tensor(
                            out=dst, in0=src, in1=rin, op=mybir.AluOpType.mult)

                exp_tiles = {}
                for G4 in range(4):
                    # scores.T: 4 heads concurrently on 4 PE row strips,
                    # each into its own psum bank (cols 0:128 of bank jj)
                    ps_s = ps_sc.tile([128, 4, 512], F32, tag="sc")
                    expg = exp_p.tile([128, 4, 128], BF16, tag="exp")
                    exp_tiles[G4] = expg
                    for jj in range(4):
                        nc.tensor.matmul(
                            ps_s[:, jj, 0:128],
                            lhsT=qkT["k"][32 * jj:32 * jj + 32, G4, cols],
                            rhs=qkT["q"][32 * jj:32 * jj + 32, G4, cols],
                            start=True, stop=True,
                            tile_position=(32 * jj, 0))
                    nc.scalar.activation(
                        out=expg[:], in_=ps_s[:, :, 0:128],
                        func=mybir.ActivationFunctionType.Exp)
                    # interleave one v-GEMM matmul of group g+2 as PE filler
                    if vnext is not None:
                        kc = G4
                        nc.tensor.matmul(
                            vnext[:],
                            lhsT=vscr_cur[:, kc],
                            rhs=wv_sb[:, kc],
                            start=(kc == 0), stop=(kc == 3))
                    if G4 > 0:
                        emit_av(G4 - 1, exp_tiles[G4 - 1])
                emit_av(3, exp_tiles[3])
                if vnext is not None:
                    psh = vnext[:].rearrange("p (h e) -> p h e", h=16)
                    vzn = vz_tiles[(g + 2) % 4]
                    nc.scalar.copy(out=vzn[0:64, :, 0:32], in_=psh[0:64])
                    nc.vector.tensor_copy(out=vzn[64:128, :, 32:64],
                                          in_=psh[64:128])

                # ---- transpose 2-group blocks -> d-major ----
                if g % 2 == 1:
                    if g % 4 == 1:
                        aoTb = aoT_p.tile([128, 4, 512], BF16, tag="aoT")
                    ps_t = ps_gemm.tile([128, 512], F32, tag="gemm")
                    ps_tv = ps_t[:].bitcast(BF16).rearrange(
                        "p (gg m t) -> p gg m t", gg=2, m=4)
                    for gg in range(2):
                        for m in range(4):
                            nc.tensor.transpose(
                                ps_tv[:, gg, m],
                                ao_tiles[gg][:, 128 * m:128 * (m + 1)],
                                ident[:])
                    half = (g % 4) // 2
                    nc.vector.tensor_copy(
                        out=aoTb[:, :, 256 * half:256 * half + 256].rearrange(
                            "p m (gg t) -> p gg m t", gg=2),
                        in_=ps_tv)

                # ---- proj GEMM per 4-group block + store ----
                if g % 4 == 3:
                    th = g // 4
                    for Mc in range(4):
                        ps_o = ps_gemm.tile([128, 512], F32, tag="gemm")
                        for m in range(4):
                            nc.tensor.matmul(
                                ps_o[:],
                                lhsT=wp_sb[:, m, 128 * Mc:128 * Mc + 128],
                                rhs=aoTb[:, m, :],
                                start=(m == 0), stop=(m == 3))
                        ost = os_p.tile([128, 512], BF16, tag="os")
                        if Mc % 2 == 0:
                            nc.vector.tensor_copy(out=ost[:], in_=ps_o[:])
                        else:
                            nc.scalar.copy(out=ost[:], in_=ps_o[:])
                        eng = nc.sync if (Mc % 2 == 0) else nc.gpsimd
                        eng.dma_start(
                            out=out_d[Mc, :, t0 + 512 * th:t0 + 512 * (th + 1)],
                            in_=ost[:])

    nc.compile()
    return nc


# ---------------- host driver ----------------

_PROG_CACHE = {}


def _get_program(CH=32, NCHUNK=4):
    key = (CH, NCHUNK)
    if key not in _PROG_CACHE:
        _PROG_CACHE[key] = build_program(CH, NCHUNK)
    return _PROG_CACHE[key]


def make_in_maps(x, wp_dict, CH=32, NCHUNK=4, n_cores=N_CORES):
    xw = window_partition(np.asarray(x, np.float32))     # (1024, 64, 512)
    nt = CH * NCHUNK * NTOK
    win_per_core = nt // NTOK
    in_maps = []
    for c in range(n_cores):
        xs = xw[c * win_per_core:(c + 1) * win_per_core].reshape(nt, DIM)
        # token reorders per chunk: xtr = (r, w, c), xtc = (c, w, r)
        x5 = xs.reshape(NCHUNK, CH, 8, 8, DIM)           # (ck, w, r, c, D)
        xr = x5.transpose(0, 2, 1, 3, 4).reshape(nt, DIM)
        xc = x5.transpose(0, 3, 1, 2, 4).reshape(nt, DIM)
        xtr = np.ascontiguousarray(xr.T).astype(ml_dtypes.bfloat16)
        xtc = np.ascontiguousarray(xc.T).astype(ml_dtypes.bfloat16)
        in_maps.append(dict(
            xtr=np.ascontiguousarray(xtr.reshape(4, 128, nt)),
            xtc=np.ascontiguousarray(xtc.reshape(4, 128, nt)),
            wqa=wp_dict["wqa"], wqb=wp_dict["wqb"],
            wka=wp_dict["wka"], wkb=wp_dict["wkb"],
            wv=wp_dict["wv"], wp=wp_dict["wp"],
        ))
    return in_maps


def _run(x, qkv_w, qkv_b, proj_w, proj_b, trace=False):
    from concourse.bass_utils import run_bass_kernel_spmd

    wp_dict = prep_weights(qkv_w, qkv_b, proj_w, proj_b)
    assert wp_dict["bias_zero"], "nonzero biases not supported by this kernel"

    nc = _get_program()
    in_maps = make_in_maps(x, wp_dict)
    res = run_bass_kernel_spmd(nc, in_maps, list(range(N_CORES)), trace=trace)

    x = np.asarray(x)
    B, H, W, C = x.shape
    outs = []
    for c in range(N_CORES):
        oT = np.asarray(res.results[c]["outT"]).astype(np.float32).reshape(DIM, NT)
        outs.append(np.ascontiguousarray(oT.T))          # (8192, 512)
    ow = np.concatenate(outs, 0).reshape(NWIN, NTOK, DIM)
    out = window_unpartition(ow, B, H, W).astype(np.float32)
    return out, res


def kernel(x, qkv_w, qkv_b, proj_w, proj_b):
    out, _ = _run(x, qkv_w, qkv_b, proj_w, proj_b, trace=False)
    return out


if __name__ == "__main__":
    build_program(4, 2)
    print("mini program built OK")
    build_program()
    print("full program built OK")


# revision 29
# speedup vs baseline: 1.1234x; 1.0120x over previous
"""Trainium2 Bass kernel for dilated window attention (nn_Dilated_attn).

Strategy (8 NeuronCores, data-parallel over the 1024 dilated windows):
 - Host regroups x into (1024 windows, 64 tok, 512) and shards 128 windows/core.
 - RoPE is folded into the QKV weights: 8 row-rotation + 8 col-rotation weight
   variants per q/k half, applied via position-sliced GEMMs. With CH=32 windows
   per chunk each weight-variant matmul has a 256-wide moving operand, so
   LDWEIGHTS is hidden under compute and the PE stays HAM-warm.
 - q.T/k.T computed d-major then DMA-scattered so head h lives on partition
   strip 32*(h%4); scores for 4 heads run concurrently via PE row tiling
   (tile_position=(32*jj, 0), K=32 quadrants).
 - AV uses exp(S).T as the stationary with per-head [v_ws0|v_ws1|ones2]
   zero-padded moving blocks: one matmul per head yields attention output and
   softmax denominators; normalize via broadcast-AP multiply on DVE.
 - PE-transpose (2-group blocks, double-buffered) -> d-major proj GEMM
   per 4-group block -> bf16 DMA out.
"""

import sys
import numpy as np
import ml_dtypes

sys.path.insert(0, "/opt/trn_rl_repo")

import concourse.bass as bass  # noqa: E402
import concourse.tile as tile  # noqa: E402
from concourse import bacc, mybir  # noqa: E402
from concourse.masks import make_identity  # noqa: E402
from contextlib import ExitStack  # noqa: E402

# ---------------- problem constants ----------------
DIM = 512
HEADS = 16
HD = 32
WH, WW = 8, 8
D0, D1 = 2, 2
TWH, TWW = 16, 16
SCALE = HD ** -0.5
N_CORES = 8
NWIN = 1024
NTOK = 64
WIN_PER_CORE = NWIN // N_CORES      # 128
NT = WIN_PER_CORE * NTOK            # 8192 tokens per core

d_half = HD // 2                    # 16
INV = 1.0 / (10000.0 ** (np.arange(0, d_half, 2, dtype=np.float64) / d_half))

BF16 = mybir.dt.bfloat16
F32 = mybir.dt.float32


# ---------------- host-side data prep ----------------

def window_partition(x):
    B, H, W, C = x.shape
    xw = x.reshape(B, H // TWH, TWH, W // TWW, TWW, C).transpose(0, 1, 3, 2, 4, 5)
    xw = xw.reshape(-1, TWH, TWW, C)
    B_ = xw.shape[0]
    xw = xw.reshape(B_, TWH // D0, D0, TWW // D1, D1, C).transpose(0, 1, 3, 2, 4, 5)
    xw = xw.reshape(B_, WH * WW, D0 * D1, C)
    return xw.transpose(0, 2, 1, 3).reshape(B_ * D0 * D1, WH * WW, C)


def window_unpartition(ow, B, H, W):
    C = ow.shape[-1]
    B_ = ow.shape[0] // (D0 * D1)
    o = ow.reshape(B_, D0 * D1, WH * WW, C).transpose(0, 2, 1, 3)
    o = o.reshape(B_, WH, WW, D0, D1, C).transpose(0, 1, 3, 2, 4, 5)
    o = o.reshape(B_, TWH, TWW, C)
    nh, nw = H // TWH, W // TWW
    o = o.reshape(B, nh, nw, TWH, TWW, C).transpose(0, 1, 3, 2, 4, 5)
    return o.reshape(B, H, W, C)


PERM32 = np.concatenate([
    np.arange(0, d_half, 2), np.arange(1, d_half, 2),
    d_half + np.arange(0, d_half, 2), d_half + np.arange(1, d_half, 2),
])


def _rot_mat(theta_vec):
    c, s = np.cos(theta_vec), np.sin(theta_vec)
    R = np.zeros((16, 16))
    R[np.arange(8), np.arange(8)] = c
    R[np.arange(8), 8 + np.arange(8)] = -s
    R[8 + np.arange(8), np.arange(8)] = s
    R[8 + np.arange(8), 8 + np.arange(8)] = c
    return R


def prep_weights(qkv_w, qkv_b, proj_w, proj_b):
    qkv_w = np.asarray(qkv_w, np.float64)
    qkv_b = np.asarray(qkv_b, np.float64)
    proj_w = np.asarray(proj_w, np.float64)
    proj_b = np.asarray(proj_b, np.float64)

    Wq = qkv_w[:DIM] * SCALE
    Wk = qkv_w[DIM:2 * DIM]
    Wv = qkv_w[2 * DIM:]
    bq = qkv_b[:DIM] * SCALE
    bk = qkv_b[DIM:2 * DIM]
    bv = qkv_b[2 * DIM:]

    perm = (np.arange(HEADS)[:, None] * HD + PERM32[None, :]).reshape(-1)
    Wq_p, bq_p = Wq[perm], bq[perm]
    Wk_p, bk_p = Wk[perm], bk[perm]

    idx = np.arange(DIM).reshape(HEADS, HD)
    a_rows = idx[:, :16].reshape(-1)
    b_rows = idx[:, 16:].reshape(-1)

    def variants(Wp, bp, rows):
        Wh, bh = Wp[rows], bp[rows]
        Ws, bs = [], []
        for t in range(8):
            R = np.kron(np.eye(HEADS), _rot_mat(t * INV))
            Ws.append(R @ Wh)
            bs.append(R @ bh)
        return np.stack(Ws), np.stack(bs)

    Wqa, bqa = variants(Wq_p, bq_p, a_rows)
    Wqb, bqb = variants(Wq_p, bq_p, b_rows)
    Wka, bka = variants(Wk_p, bk_p, a_rows)
    Wkb, bkb = variants(Wk_p, bk_p, b_rows)

    proj_b_eff = proj_w @ bv + proj_b
    bias_zero = (np.abs(np.concatenate([bqa, bqb, bka, bkb], None)).max() == 0.0
                 and np.abs(proj_b_eff).max() == 0.0)

    def pack_lhsT(Wvar):  # (8, 256, 512) -> (8, 4, 128, 256) bf16
        WT = Wvar.transpose(0, 2, 1)                # (8, 512, 256)
        return np.ascontiguousarray(
            WT.reshape(8, 4, 128, 256)).astype(ml_dtypes.bfloat16)

    return dict(
        wqa=pack_lhsT(Wqa), wqb=pack_lhsT(Wqb),
        wka=pack_lhsT(Wka), wkb=pack_lhsT(Wkb),
        wv=np.ascontiguousarray(Wv.T.reshape(4, 128, 512)).astype(ml_dtypes.bfloat16),
        wp=np.ascontiguousarray(proj_w.T.reshape(4, 128, 512)).astype(ml_dtypes.bfloat16),
        proj_b_eff=proj_b_eff.astype(np.float32),
        bias_zero=bias_zero,
    )


# ---------------- device program ----------------

def build_program(CH=32, NCHUNK=4, debug_stop=None):
    """One-core SPMD program. CH windows per chunk, NCHUNK chunks.
    debug_stop: one of qkgemm|scatter|vprime|scores|av|trans to truncate
    the pipeline after that stage and DMA the stage output to a `dbg` tensor."""
    nt = CH * NCHUNK * NTOK        # tokens per core
    TPC = CH * NTOK                # tokens per chunk
    NG = CH // 2                   # 2-window groups per chunk

    nc = bacc.Bacc(trn_type="TRN2", target_bir_lowering=False, debug=False)

    # xtr: tokens ordered (r, w, c) per chunk -> a-half moving operands are
    # contiguous 256-col blocks. xtc: (c, w, r) order for the b-half.
    # Strided moving APs cost ~2x per column, so contiguity is critical.
    xtr_d = nc.dram_tensor("xtr", [4, 128, nt], BF16, kind="ExternalInput").ap()
    xtc_d = nc.dram_tensor("xtc", [4, 128, nt], BF16, kind="ExternalInput").ap()
    w_d = {}
    for nm in ("wqa", "wqb", "wka", "wkb"):
        w_d[nm] = nc.dram_tensor(nm, [8, 4, 128, 256], BF16,
                                 kind="ExternalInput").ap()
    wv_d = nc.dram_tensor("wv", [4, 128, 512], BF16, kind="ExternalInput").ap()
    wp_d = nc.dram_tensor("wp", [4, 128, 512], BF16, kind="ExternalInput").ap()
    out_d = nc.dram_tensor("outT", [4, 128, nt], BF16, kind="ExternalOutput").ap()

    with tile.TileContext(nc) as tc, ExitStack() as ctx:
        const_p = ctx.enter_context(tc.tile_pool(name="const", bufs=1))
        w_p = ctx.enter_context(tc.tile_pool(name="weights", bufs=1))
        xt_p = ctx.enter_context(tc.tile_pool(name="xt", bufs=2))
        stag_p = ctx.enter_context(tc.tile_pool(name="stag", bufs=2))
        qkT_p = ctx.enter_context(tc.tile_pool(name="qkT", bufs=1))
        vp_p = ctx.enter_context(tc.tile_pool(name="vp", bufs=1))
        exp_p = ctx.enter_context(tc.tile_pool(name="exp", bufs=3))
        ao_p = ctx.enter_context(tc.tile_pool(name="ao", bufs=4))
        aoT_p = ctx.enter_context(tc.tile_pool(name="aoT", bufs=2))
        rc_p = ctx.enter_context(tc.tile_pool(name="rc", bufs=2))
        os_p = ctx.enter_context(tc.tile_pool(name="os", bufs=2))
        vs_p = ctx.enter_context(tc.tile_pool(name="vs", bufs=3))

        ps_gemm = ctx.enter_context(tc.tile_pool(name="ps_gemm", bufs=2, space="PSUM"))
        ps_sc = ctx.enter_context(tc.tile_pool(name="ps_sc", bufs=1, space="PSUM"))
        ps_av = ctx.enter_context(tc.tile_pool(name="ps_av", bufs=2, space="PSUM"))

        ident = const_p.tile([128, 128], BF16)
        make_identity(nc, ident[:])

        # resident weights
        w_sb = {}
        for nm in ("wqa", "wqb", "wka", "wkb"):
            t = w_p.tile([128, 8, 4, 256], BF16, tag=nm)
            nc.sync.dma_start(out=t[:], in_=w_d[nm].rearrange("v k p m -> p v k m"))
            w_sb[nm] = t
        wv_sb = w_p.tile([128, 4, 512], BF16, tag="wv")
        nc.sync.dma_start(out=wv_sb[:], in_=wv_d.rearrange("k p n -> p k n"))
        wp_sb = w_p.tile([128, 4, 512], BF16, tag="wp")
        nc.sync.dma_start(out=wp_sb[:], in_=wp_d.rearrange("k p n -> p k n"))

        # persistent zero-padded v tiles (zeros/ones written once, data per g).
        # Per head h a 66-col block [v_ws0(32) | v_ws1(32) | ones(2)] so AV is
        # a single N=66 matmul per head (denominators fused via ones cols).
        vz_tiles = []
        for i in range(4):
            vzt = vp_p.tile([128, 16, 66], BF16, tag=f"vp{i}")
            nc.gpsimd.memset(vzt[64:128, :, 0:32], 0.0)
            nc.gpsimd.memset(vzt[0:64, :, 32:64], 0.0)
            nc.gpsimd.memset(vzt[:, :, 64:66], 0.0)
            nc.gpsimd.memset(vzt[0:64, :, 64:65], 1.0)
            nc.gpsimd.memset(vzt[64:128, :, 65:66], 1.0)
            vz_tiles.append(vzt)

        # persistent qT/kT tiles: head h = 4*hg + jj lives on partition strip
        # 32*jj (+16 for the b-half) at free block hg. Fixed 32-aligned strips
        # let scores run 4 heads concurrently via PE row tiling.
        qkT = {}
        for T in "qk":
            qkT[T] = qkT_p.tile([128, 4, TPC], BF16, tag=T + "T",
                                name=T + "T_tile")

        def emit_vgemm(ck, g):
            # stationary (weights) APs must be single-free-dim: gather the
            # group's 128 tokens from xtr into a contiguous scratch (gpsimd,
            # otherwise idle) before using them as the v-GEMM stationary.
            t0 = ck * TPC
            xr4 = xt_tiles[ck % 2][:].rearrange(
                "p k (r w c) -> p k r w c", r=8, w=CH)
            scr = vs_p.tile([128, 4, 128], BF16, tag="vscr")
            nc.gpsimd.tensor_copy(
                out=scr[:].rearrange("p k (w r c) -> p k r w c", w=2, r=8),
                in_=xr4[:, :, :, 2 * g:2 * g + 2, :])
            ps = ps_gemm.tile([128, 512], F32, tag="gemm")
            for kc in range(4):
                nc.tensor.matmul(
                    ps[:],
                    lhsT=scr[:, kc],
                    rhs=wv_sb[:, kc],
                    start=(kc == 0), stop=(kc == 3))
            psh = ps[:].rearrange("p (h e) -> p h e", h=16)
            vzg = vz_tiles[g % 4]
            nc.scalar.copy(out=vzg[0:64, :, 0:32], in_=psh[0:64])
            nc.vector.tensor_copy(out=vzg[64:128, :, 32:64], in_=psh[64:128])

        xt_tiles = {}

        for ck in range(NCHUNK):
            t0 = ck * TPC

            # ---- load x.T chunk (both token orders) ----
            xt_t = xt_p.tile([128, 4, TPC], BF16, tag="xtr")
            xt_tiles[ck % 2] = xt_t
            nc.sync.dma_start(
                out=xt_t[:],
                in_=xtr_d[:, :, t0:t0 + TPC].rearrange("k p t -> p k t"))
            xtc_t = xt_p.tile([128, 4, TPC], BF16, tag="xtc", bufs=1)
            nc.gpsimd.dma_start(
                out=xtc_t[:],
                in_=xtc_d[:, :, t0:t0 + TPC].rearrange("k p t -> p k t"))

            # ---- q/k GEMMs (rope folded) into staging, then scatter ----
            # Moving operand for variant v8 is a contiguous 256-col block of
            # xtr (a-half, token order r,w,c) or xtc (b-half, order c,w,r).
            for Ti, T in enumerate("qk"):
                for Hi, half in enumerate("ab"):
                    wt = w_sb["w" + T + half]
                    xsrc = xt_t if half == "a" else xtc_t
                    stag = stag_p.tile([128, 2, TPC], BF16, tag="stag")
                    for Mc in range(2):
                        if half == "a":
                            dst4 = stag[:, Mc].rearrange(
                                "p (w r c) -> p r w c", w=CH, r=8, c=8)
                        else:
                            dst4 = stag[:, Mc].rearrange(
                                "p (w r c) -> p c w r", w=CH, r=8, c=8)
                        NW = CH * 8
                        for vp in range(4):
                            ps = ps_gemm.tile([128, 512], F32, tag="gemm")
                            for vv in range(2):
                                v8 = 2 * vp + vv
                                for kc in range(4):
                                    nc.tensor.matmul(
                                        ps[:, NW * vv:NW * (vv + 1)],
                                        lhsT=wt[:, v8, kc, 128 * Mc:128 * Mc + 128],
                                        rhs=xsrc[:, kc,
                                                 NW * v8:NW * (v8 + 1)],
                                        start=(vv == 0 and kc == 0),
                                        stop=(vv == 1 and kc == 3))
                            if half == "a":
                                # dst runs of 8 (c contiguous): fine on ACT
                                nc.scalar.copy(
                                    out=dst4[:, 2 * vp:2 * vp + 2],
                                    in_=ps[:, 0:2 * NW].rearrange(
                                        "p (v w c) -> p v w c", v=2, w=CH))
                            else:
                                # b-half dst is c-strided; put the variant
                                # pair innermost (2-elem runs instead of 1)
                                # and alternate engines across vp pairs.
                                bdst = stag[:, Mc].rearrange(
                                    "p (w r c) -> p w r c", w=CH, r=8)[
                                    :, :, :, 2 * vp:2 * vp + 2]
                                bsrc = ps[:, 0:2 * NW].rearrange(
                                    "p (v w r) -> p w r v", v=2, w=CH)
                                hw = CH // 2
                                nc.vector.tensor_copy(
                                    out=bdst[:, 0:hw], in_=bsrc[:, 0:hw])
                                nc.scalar.copy(
                                    out=bdst[:, hw:CH], in_=bsrc[:, hw:CH])

                    # scatter this (T, half) into qkT strips
                    qT = qkT[T]
                    for hl in range(8):
                        src = stag[16 * hl:16 * hl + 16, :, :]
                        dstp = 32 * (hl % 4) + 16 * Hi
                        dst = qT[dstp:dstp + 16].rearrange(
                            "p (m hg) t -> p m hg t", m=2)[:, :, hl // 4, :]
                        eng = nc.sync if (hl % 2 == 0) else nc.gpsimd
                        eng.dma_start(out=dst, in_=src)
            if debug_stop == "scatter":
                dbg = nc.dram_tensor("dbg", [2, 128, 4, TPC], BF16,
                                     kind="ExternalOutput").ap()
                nc.sync.dma_start(out=dbg[0], in_=qkT["q"][:])
                nc.sync.dma_start(out=dbg[1], in_=qkT["k"][:])
                break

            # ---- per 2-window group: v GEMM + attention ----
            # Scores for 4 heads run concurrently on the 4 PE row strips;
            # each row-tile must drain to its OWN psum bank (same-bank
            # concurrent drains are a fatal HW error), hence the
            # [128, 4, 512] 4-bank scores tile. v-GEMM matmuls are
            # interleaved between attention groups so the PE keeps ahead
            # of the exp (ACT) latency on the shared scores banks.
            ao_tiles = {}
            aoTb = None
            for g in range(NG):
                if g == 0:
                    emit_vgemm(ck, 0)
                    emit_vgemm(ck, 1)
                vzg = vz_tiles[g % 4]
                cols = slice(128 * g, 128 * (g + 1))

                rcpg = rc_p.tile([128, 4, 4, 2], F32, tag="rc")
                aog = ao_p.tile([128, 512], BF16, tag="ao")
                ao_tiles[g % 2] = aog

                vnext = None
                vscr_cur = None
                if g + 2 < NG:
                    xr4 = xt_tiles[ck % 2][:].rearrange(
                        "p k (r w c) -> p k r w c", r=8, w=CH)
                    vscr_cur = vs_p.tile([128, 4, 128], BF16, tag="vscr")
                    nc.gpsimd.tensor_copy(
                        out=vscr_cur[:].rearrange(
                            "p k (w r c) -> p k r w c", w=2, r=8),
                        in_=xr4[:, :, :, 2 * (g + 2):2 * (g + 2) + 2, :])
                    vnext = ps_gemm.tile([128, 512], F32, tag="gemm")

                def emit_av(G4, expg):
                    ps_a = ps_av.tile([128, 4, 66], F32, tag="av")
                    for jj in range(4):
                        h = 4 * G4 + jj
                        nc.tensor.matmul(
                            ps_a[:, jj],
                            lhsT=expg[:, jj],
                            rhs=vzg[:, h, :],
                            start=(jj == 0), stop=(jj == 3))
                    # normalize: recip of denominators, scale valid halves
                    nc.vector.reciprocal(
                        out=rcpg[:, G4], in_=ps_a[:, :, 64:66])
                    for ws in range(2):
                        src = ps_a[64 * ws:64 * ws + 64, :, 32 * ws:32 * ws + 32]
                        rin = rcpg[64 * ws:64 * ws + 64, G4, :, ws:ws + 1] \
                            .broadcast_to((64, 4, 32))
                        dst = aog[64 * ws:64 * ws + 64,
                                  128 * G4:128 * (G4 + 1)].rearrange(
                            "p (j e) -> p j e", j=4)
                        nc.vector.tensor_tensor(
                            out=dst, in0=src, in1=rin, op=mybir.AluOpType.mult)

                exp_tiles = {}
                for G4 in range(4):
                    # scores.T: 4 heads concurrently on 4 PE row strips,
                    # each into its own psum bank (cols 0:128 of bank jj)
                    ps_s = ps_sc.tile([128, 4, 512], F32, tag="sc")
                    expg = exp_p.tile([128, 4, 128], BF16, tag="exp")
                    exp_tiles[G4] = expg
                    for jj in range(4):
                        nc.tensor.matmul(
                            ps_s[:, jj, 0:128],
                            lhsT=qkT["k"][32 * jj:32 * jj + 32, G4, cols],
                            rhs=qkT["q"][32 * jj:32 * jj + 32, G4, cols],
                            start=True, stop=True,
                            tile_position=(32 * jj, 0))
                    nc.scalar.activation(
                        out=expg[:], in_=ps_s[:, :, 0:128],
                        func=mybir.ActivationFunctionType.Exp)
                    # interleave one v-GEMM matmul of group g+2 as PE filler
                    if vnext is not None:
                        kc = G4
                        nc.tensor.matmul(
                            vnext[:],
                            lhsT=vscr_cur[:, kc],
                            rhs=wv_sb[:, kc],
                            start=(kc == 0), stop=(kc == 3))
                    if G4 > 0:
                        emit_av(G4 - 1, exp_tiles[G4 - 1])
                emit_av(3, exp_tiles[3])
                if vnext is not None:
                    psh = vnext[:].rearrange("p (h e) -> p h e", h=16)
                    vzn = vz_tiles[(g + 2) % 4]
                    nc.scalar.copy(out=vzn[0:64, :, 0:32], in_=psh[0:64])
                    nc.vector.tensor_copy(out=vzn[64:128, :, 32:64],
                                          in_=psh[64:128])

                # ---- transpose 2-group blocks -> d-major ----
                if g % 2 == 1:
                    if g % 4 == 1:
                        aoTb = aoT_p.tile([128, 4, 512], BF16, tag="aoT")
                    ps_t = ps_gemm.tile([128, 512], F32, tag="gemm")
                    ps_tv = ps_t[:].bitcast(BF16).rearrange(
                        "p (gg m t) -> p gg m t", gg=2, m=4)
                    for gg in range(2):
                        for m in range(4):
                            nc.tensor.transpose(
                                ps_tv[:, gg, m],
                                ao_tiles[gg][:, 128 * m:128 * (m + 1)],
                                ident[:])
                    half = (g % 4) // 2
                    nc.vector.tensor_copy(
                        out=aoTb[:, :, 256 * half:256 * half + 256].rearrange(
                            "p m (gg t) -> p gg m t", gg=2),
                        in_=ps_tv)

                # ---- proj GEMM per 4-group block + store ----
                if g % 4 == 3:
                    th = g // 4
                    for Mc in range(4):
                        ps_o = ps_gemm.tile([128, 512], F32, tag="gemm")
                        for m in range(4):
                            nc.tensor.matmul(
                                ps_o[:],
                                lhsT=wp_sb[:, m, 128 * Mc:128 * Mc + 128],
                                rhs=aoTb[:, m, :],
                                start=(m == 0), stop=(m == 3))
                        ost = os_p.tile([128, 512], BF16, tag="os")
                        if Mc % 2 == 0:
                            nc.vector.tensor_copy(out=ost[:], in_=ps_o[:])
                        else:
                            nc.scalar.copy(out=ost[:], in_=ps_o[:])
                        eng = nc.sync if (Mc % 2 == 0) else nc.gpsimd
                        eng.dma_start(
                            out=out_d[Mc, :, t0 + 512 * th:t0 + 512 * (th + 1)],
                            in_=ost[:])

    nc.compile()
    return nc


# ---------------- host driver ----------------

_PROG_CACHE = {}


def _get_program(CH=32, NCHUNK=4):
    key = (CH, NCHUNK)
    if key not in _PROG_CACHE:
        _PROG_CACHE[key] = build_program(CH, NCHUNK)
    return _PROG_CACHE[key]


def make_in_maps(x, wp_dict, CH=32, NCHUNK=4, n_cores=N_CORES):
    xw = window_partition(np.asarray(x, np.float32))     # (1024, 64, 512)
    nt = CH * NCHUNK * NTOK
    win_per_core = nt // NTOK
    in_maps = []
    for c in range(n_cores):
        xs = xw[c * win_per_core:(c + 1) * win_per_core].reshape(nt, DIM)
        # token reorders per chunk: xtr = (r, w, c), xtc = (c, w, r)
        x5 = xs.reshape(NCHUNK, CH, 8, 8, DIM)           # (ck, w, r, c, D)
        xr = x5.transpose(0, 2, 1, 3, 4).reshape(nt, DIM)
        xc = x5.transpose(0, 3, 1, 2, 4).reshape(nt, DIM)
        xtr = np.ascontiguousarray(xr.T).astype(ml_dtypes.bfloat16)
        xtc = np.ascontiguousarray(xc.T).astype(ml_dtypes.bfloat16)
        in_maps.append(dict(
            xtr=np.ascontiguousarray(xtr.reshape(4, 128, nt)),
            xtc=np.ascontiguousarray(xtc.reshape(4, 128, nt)),
            wqa=wp_dict["wqa"], wqb=wp_dict["wqb"],
            wka=wp_dict["wka"], wkb=wp_dict["wkb"],
            wv=wp_dict["wv"], wp=wp_dict["wp"],
        ))
    return in_maps


def _run(x, qkv_w, qkv_b, proj_w, proj_b, trace=False):
    from concourse.bass_utils import run_bass_kernel_spmd

    wp_dict = prep_weights(qkv_w, qkv_b, proj_w, proj_b)
    assert wp_dict["bias_zero"], "nonzero biases not supported by this kernel"

    nc = _get_program()
    in_maps = make_in_maps(x, wp_dict)
    res = run_bass_kernel_spmd(nc, in_maps, list(range(N_CORES)), trace=trace)

    x = np.asarray(x)
    B, H, W, C = x.shape
    outs = []
    for c in range(N_CORES):
        oT = np.asarray(res.results[c]["outT"]).astype(np.float32).reshape(DIM, NT)
        outs.append(np.ascontiguousarray(oT.T))          # (8192, 512)
    ow = np.concatenate(outs, 0).reshape(NWIN, NTOK, DIM)
    out = window_unpartition(ow, B, H, W).astype(np.float32)
    return out, res


def kernel(x, qkv_w, qkv_b, proj_w, proj_b):
    out, _ = _run(x, qkv_w, qkv_b, proj_w, proj_b, trace=False)
    return out


if __name__ == "__main__":
    build_program(4, 2)
    print("mini program built OK")
    build_program()
    print("full program built OK")
